# revision 1
# baseline (speedup 1.0000x reference)
"""GRNN regressor on 8 TRN2 NeuronCores.

Math: w[b,n] = exp(-(||x_b||^2 + ||t_n||^2 - 2 x_b.t_n)/2); out[b] = (w@y)/(w@1).

Strategy (matches the sharding hint): X_train/y_train sharded over N across
8 cores; x replicated. Per core, one matmul with an augmented feature dim
(K=66: 64 features + both squared-norm terms) produces -dist^2/2 directly in
PSUM with train-points on partitions; ScalarE Exp turns it into weights; a
second PSUM-accumulating matmul against [y, 1] contracts over train points,
yielding per-core partial [numerator; denominator] of shape [2, 4096].
The all-reduce over cores plus the final divide happen on host (32KB/core).
"""

import numpy as np

B, F, N, P = 4096, 64, 100000, 8
NS = N // P            # 12500 train points per core
NB = 128               # train-point block (PSUM partitions)
NSP = ((NS + NB - 1) // NB) * NB   # 12544 padded
NBLK = NSP // NB       # 98
BBLK = 512             # query block (moving free dim / PSUM bank)
K = F + 2              # augmented contraction dim

_cache = {}


def _build():
    import concourse.bacc as bacc
    import concourse.mybir as mybir
    import concourse.tile as tile

    dt = mybir.dt.float32
    nc = bacc.Bacc("TRN2", target_bir_lowering=False, debug=False)
    xa_d = nc.dram_tensor("xa", [K, B], dt, kind="ExternalInput")
    ta_d = nc.dram_tensor("ta", [K, NSP], dt, kind="ExternalInput")
    y1_d = nc.dram_tensor("y1", [NB, 2 * NBLK], dt, kind="ExternalInput")
    out_d = nc.dram_tensor("out", [2, B], dt, kind="ExternalOutput")

    with tile.TileContext(nc) as tc:
        with (
            tc.tile_pool(name="const", bufs=1) as cpool,
            tc.tile_pool(name="w", bufs=3) as wpool,
            tc.tile_pool(name="res", bufs=2) as rpool,
            tc.tile_pool(name="ps", bufs=3, space="PSUM") as spool,
            tc.tile_pool(name="pacc", bufs=2, space="PSUM") as apool,
        ):
            xa = cpool.tile([K, B], dt)
            ta = cpool.tile([K, NSP], dt)
            y1 = cpool.tile([NB, 2 * NBLK], dt)
            zb = cpool.tile([NB, 1], dt)
            nc.sync.dma_start(xa[:], xa_d[:])
            nc.sync.dma_start(ta[:], ta_d[:])
            nc.sync.dma_start(y1[:], y1_d[:])
            nc.gpsimd.memset(zb[:], 0.0)

            for b in range(B // BBLK):
                acc = apool.tile([2, BBLK], dt)
                xsl = xa[:, b * BBLK : (b + 1) * BBLK]
                for ni in range(NBLK):
                    s = spool.tile([NB, BBLK], dt)
                    nc.tensor.matmul(
                        s[:], ta[:, ni * NB : (ni + 1) * NB], xsl,
                        start=True, stop=True,
                    )
                    w = wpool.tile([NB, BBLK], dt)
                    nc.scalar.activation(
                        w[:], s[:], mybir.ActivationFunctionType.Exp, bias=zb[:]
                    )
                    nc.tensor.matmul(
                        acc[:], y1[:, 2 * ni : 2 * ni + 2], w[:],
                        start=(ni == 0), stop=(ni == NBLK - 1),
                    )
                res = rpool.tile([2, BBLK], dt)
                nc.vector.tensor_copy(res[:], acc[:])
                nc.sync.dma_start(out_d[:, b * BBLK : (b + 1) * BBLK], res[:])

    nc.compile()
    return nc


def kernel(x, X_train, y_train):
    from concourse.bass_utils import run_bass_kernel_spmd

    x = np.asarray(x, np.float32)
    X_train = np.asarray(X_train, np.float32)
    y_train = np.asarray(y_train, np.float32)

    xa = np.empty((K, B), np.float32)
    xa[:F] = x.T
    xa[F] = -0.5 * np.sum(x * x, axis=1)
    xa[F + 1] = 1.0

    in_maps = []
    for c in range(P):
        t = X_train[c * NS : (c + 1) * NS]
        ta = np.zeros((K, NSP), np.float32)
        ta[:F, :NS] = t.T
        ta[F, :] = 1.0
        ta[F + 1, :NS] = -0.5 * np.sum(t * t, axis=1)
        ta[F + 1, NS:] = -1e30  # pad columns get weight exp(-inf) = 0
        y1 = np.zeros((NB, 2 * NBLK), np.float32)
        yc = np.zeros(NSP, np.float32)
        yc[:NS] = y_train[c * NS : (c + 1) * NS]
        y1[:, 0::2] = yc.reshape(NBLK, NB).T
        y1[:, 1::2] = 1.0
        in_maps.append({"xa": xa, "ta": ta, "y1": y1})

    if "nc" not in _cache:
        _cache["nc"] = _build()
    res = run_bass_kernel_spmd(_cache["nc"], in_maps, core_ids=list(range(P)))
    parts = np.stack([np.asarray(r["out"]) for r in res.results])  # [P, 2, B]
    tot = parts.sum(axis=0, dtype=np.float64)
    return (tot[0] / tot[1]).astype(np.float32)



# revision 3
# speedup vs baseline: 1.7382x; 1.7382x over previous
"""GRNN regressor on 8 TRN2 NeuronCores.

Math: w[b,n] = exp(-(||x_b||^2 + ||t_n||^2 - 2 x_b.t_n)/2); out[b] = (w@y)/(w@1).

Strategy: X_train/y_train sharded over N across 8 cores; x replicated.
The per-query factor exp(-||x_b||^2/2) multiplies numerator and denominator
identically, so it cancels in the ratio and is dropped entirely. Per core,
one bf16 matmul with K=66 (64 features + hi/lo split of -||t||^2/2 against
ones) produces s = x.t - ||t||^2/2 in PSUM with train-points on partitions;
ScalarE Exp over multi-bank PSUM groups (free dim 2048/1536 to amortize the
~172-cycle PSUM access overhead) produces bf16 weights; a second
PSUM-accumulating bf16 matmul against [y, 1] contracts over train points,
yielding per-core partials [numerator; denominator] of shape [2, 4096].
The all-reduce over cores plus the final divide happen on host (32KB/core).
"""

import numpy as np

B, F, N, P = 4096, 64, 100000, 8
NS = N // P            # 12500 train points per core
NB = 128               # train-point block (PSUM partitions)
NSP = ((NS + NB - 1) // NB) * NB   # 12544 padded
NBLK = NSP // NB       # 98
BBLK = 512             # query block (moving free dim / PSUM bank)
K = F + 2              # 64 features + hi/lo point-norm rows
GA, GB = 4, 3          # point-blocks per PSUM group (4-bank + 3-bank)
NGRP = NBLK // (GA + GB)  # 14

_cache = {}


def build_nc(repeat=1):
    import concourse.bacc as bacc
    import concourse.mybir as mybir
    import concourse.tile as tile

    f32 = mybir.dt.float32
    bf16 = mybir.dt.bfloat16
    nc = bacc.Bacc("TRN2", target_bir_lowering=False, debug=False)
    xa_d = nc.dram_tensor("xa", [K, B], bf16, kind="ExternalInput")
    ta_d = nc.dram_tensor("ta", [K, NSP], bf16, kind="ExternalInput")
    y1_d = nc.dram_tensor("y1", [NB, 2 * NBLK], bf16, kind="ExternalInput")
    out_d = nc.dram_tensor("out", [2, B], f32, kind="ExternalOutput")

    with tile.TileContext(nc) as tc:
        with (
            tc.tile_pool(name="const", bufs=1) as cpool,
            tc.tile_pool(name="wa", bufs=2) as wpa,
            tc.tile_pool(name="wb", bufs=2) as wpb,
            tc.tile_pool(name="res", bufs=2) as rpool,
            tc.tile_pool(name="sa", bufs=1, space="PSUM") as spa,
            tc.tile_pool(name="sb", bufs=1, space="PSUM") as spb,
            tc.tile_pool(name="pacc", bufs=1, space="PSUM") as apool,
        ):
            xa = cpool.tile([K, B], bf16)
            ta = cpool.tile([K, NSP], bf16)
            y1 = cpool.tile([NB, 2 * NBLK], bf16)
            nc.sync.dma_start(xa[:], xa_d[:])
            nc.sync.dma_start(ta[:], ta_d[:])
            nc.sync.dma_start(y1[:], y1_d[:])

            def emit_sec(acc, pending):
                # second matmuls for a (w_tile, n0, count) batch from an
                # earlier group, so they queue on PE behind the next group's
                # main matmuls instead of stalling them
                w, n0, cnt = pending
                for j in range(cnt):
                    ni = n0 + j
                    nc.tensor.matmul(
                        acc[:], y1[:, 2 * ni : 2 * ni + 2],
                        w[:, j * BBLK : (j + 1) * BBLK],
                        start=(ni == 0), stop=(ni == NBLK - 1),
                    )

            for _ in range(repeat):
                for b in range(B // BBLK):
                    acc = apool.tile([2, BBLK], f32)
                    xsl = xa[:, b * BBLK : (b + 1) * BBLK]
                    pending = []
                    for g in range(NGRP):
                        n0 = g * (GA + GB)
                        sA = spa.tile([NB, GA * BBLK], f32)
                        for j in range(GA):
                            ni = n0 + j
                            nc.tensor.matmul(
                                sA[:, j * BBLK : (j + 1) * BBLK],
                                ta[:, ni * NB : (ni + 1) * NB], xsl,
                                start=True, stop=True,
                            )
                        sB = spb.tile([NB, GB * BBLK], f32)
                        for j in range(GB):
                            ni = n0 + GA + j
                            nc.tensor.matmul(
                                sB[:, j * BBLK : (j + 1) * BBLK],
                                ta[:, ni * NB : (ni + 1) * NB], xsl,
                                start=True, stop=True,
                            )
                        wA = wpa.tile([NB, GA * BBLK], bf16)
                        nc.scalar.activation(
                            wA[:], sA[:], mybir.ActivationFunctionType.Exp
                        )
                        wB = wpb.tile([NB, GB * BBLK], bf16)
                        nc.scalar.activation(
                            wB[:], sB[:], mybir.ActivationFunctionType.Exp
                        )
                        for p in pending:
                            emit_sec(acc, p)
                        pending = [(wA, n0, GA), (wB, n0 + GA, GB)]
                    for p in pending:
                        emit_sec(acc, p)
                    res = rpool.tile([2, BBLK], f32)
                    nc.vector.tensor_copy(res[:], acc[:])
                    nc.sync.dma_start(out_d[:, b * BBLK : (b + 1) * BBLK], res[:])

    nc.compile()
    return nc


def _prep_inputs(x, X_train, y_train):
    from ml_dtypes import bfloat16

    x = np.asarray(x, np.float32)
    X_train = np.asarray(X_train, np.float32)
    y_train = np.asarray(y_train, np.float32)

    xa = np.ones((K, B), np.float32)
    xa[:F] = x.T
    xa_b = xa.astype(bfloat16)

    in_maps = []
    for c in range(P):
        t = X_train[c * NS : (c + 1) * NS]
        tn = -0.5 * np.sum(t * t, axis=1, dtype=np.float32)
        ta = np.zeros((K, NSP), np.float32)
        ta[:F, :NS] = t.T
        ta[F, :NS] = tn
        ta[F, NS:] = -60000.0  # pad columns get weight exp(-big) = 0
        ta_b = ta.astype(bfloat16)
        # low part of the norm row so bf16 keeps the exponent exact to ~2^-17
        ta_b[F + 1, :NS] = (tn - ta_b[F, :NS].astype(np.float32)).astype(bfloat16)

        y1 = np.zeros((NB, 2 * NBLK), np.float32)
        yc = np.zeros(NSP, np.float32)
        yc[:NS] = y_train[c * NS : (c + 1) * NS]
        y1[:, 0::2] = yc.reshape(NBLK, NB).T
        y1[:, 1::2] = 1.0
        in_maps.append({"xa": xa_b, "ta": ta_b, "y1": y1.astype(bfloat16)})
    return in_maps


def kernel(x, X_train, y_train):
    from concourse.bass_utils import run_bass_kernel_spmd

    in_maps = _prep_inputs(x, X_train, y_train)
    if "nc" not in _cache:
        _cache["nc"] = build_nc()
    res = run_bass_kernel_spmd(_cache["nc"], in_maps, core_ids=list(range(P)))
    _cache["last_results"] = res
    parts = np.stack([np.asarray(r["out"]) for r in res.results])  # [P, 2, B]
    tot = parts.sum(axis=0, dtype=np.float64)
    return (tot[0] / tot[1]).astype(np.float32)
